# revision 6
# baseline (speedup 1.0000x reference)
"""Trainium2 Bass kernel for nn_BoundMemUpdate (spiking membrane update).

Computes, for x:[T,B,D], W:[D,D], b:[D]:
    mm[t] = x[t] @ W.T + b
    m[t] = mm[t] + m[t-1] * (1 - s[t-1]) * 0.5
    s[t] = (m[t] >= 1.0)
Returns (m, s), each [T, B, D] float32.

Sharding: output-dim (D_out) sharded 8 ways across cores (512 each);
x replicated, W/b sharded by rows. The recurrence is per-neuron
elementwise, so no cross-core communication.

Device matmul runs as a 3-term float32r split (hi/lo, 11 explicit
mantissa bits). The TRN2 PE's float32r multiplier is exact for operands
with <=11 explicit mantissa bits (measured), so
    x@W ~= xh@Wh + xh@Wl + xl@Wh
carries full fp32-class accuracy (dropped xl@Wl term ~2^-24 relative)
at 1 cycle/row instead of fp32's 4.

Host side does layout prep only (transpose + hi/lo split), which is not
part of device execution time.
"""
import os
import numpy as np

import concourse.bass as bass
import concourse.mybir as mybir
from concourse import bacc
from concourse.tile import TileContext
from concourse.bass_utils import run_bass_kernel_spmd

T, B, D = 8, 256, 4096
N_CORES = 8
O_SHARD = D // N_CORES  # 512
KT = D // 128  # 32 k-tiles
MT = B // 128  # 2 m-tiles
ALPHA = 0.5
M_TH = 1.0

# "f32r3": 3-term float32r hi/lo split matmul (fast, fp32-class accuracy)
# "f32":   plain fp32 matmul (4 cycles/row on PE)
MODE = os.environ.get("BMU_MODE", "f32r3")

_cache = {}


def _round_to_bits(a: np.ndarray, explicit_bits: int) -> np.ndarray:
    """Round-to-nearest-even fp32 -> fp32 with `explicit_bits` explicit
    mantissa bits. Carry into the exponent is handled by integer add."""
    shift = np.uint32(23 - explicit_bits)
    u = a.view(np.uint32)
    half = np.uint32((1 << (23 - explicit_bits - 1)) - 1)
    lsb = (u >> shift) & np.uint32(1)
    u2 = (u + half + lsb) & ~np.uint32((1 << (23 - explicit_bits)) - 1)
    return u2.view(np.float32)


def _build_kernel(mode: str):
    nc = bacc.Bacc("TRN2", target_bir_lowering=False, debug=False,
                   num_devices=N_CORES)
    f32 = mybir.dt.float32
    dt_mm = mybir.dt.float32r if mode == "f32r3" else f32
    split = mode == "f32r3"

    # Per-core DRAM tensors. x transposed to [T, D_in, B] on host.
    xh_d = nc.dram_tensor("xh", [T, D, B], dt_mm, kind="ExternalInput").ap()
    if split:
        xl_d = nc.dram_tensor("xl", [T, D, B], dt_mm, kind="ExternalInput").ap()
    wh_d = nc.dram_tensor("wh", [D, O_SHARD], dt_mm, kind="ExternalInput").ap()
    if split:
        wl_d = nc.dram_tensor("wl", [D, O_SHARD], dt_mm, kind="ExternalInput").ap()
    b_d = nc.dram_tensor("bias", [O_SHARD], dt_mm, kind="ExternalInput").ap()
    ones_d = nc.dram_tensor("ones", [128], dt_mm, kind="ExternalInput").ap()
    m_d = nc.dram_tensor("m_out", [T, B, O_SHARD], f32, kind="ExternalOutput").ap()
    s_d = nc.dram_tensor("s_out", [T, B, O_SHARD], f32, kind="ExternalOutput").ap()

    with TileContext(nc) as tc:
        with tc.tile_pool(name="wpool", bufs=1) as wpool, \
             tc.tile_pool(name="xpool", bufs=8) as xpool, \
             tc.tile_pool(name="cpool", bufs=1) as cpool, \
             tc.tile_pool(name="mpool", bufs=4) as mpool, \
             tc.tile_pool(name="spool", bufs=4) as spool, \
             tc.tile_pool(name="upool", bufs=2) as upool, \
             tc.tile_pool(name="psum", bufs=4, space="PSUM") as psum_pool:

            # Resident weights: 32 k-tiles of [128, O_SHARD] (hi and lo).
            wh_t, wl_t = [], []
            for k in range(KT):
                wht = wpool.tile([128, O_SHARD], dt_mm, name=f"wh{k}")
                nc.sync.dma_start(out=wht, in_=wh_d[k * 128:(k + 1) * 128, :])
                wh_t.append(wht)
                if split:
                    wlt = wpool.tile([128, O_SHARD], dt_mm, name=f"wl{k}")
                    nc.sync.dma_start(out=wlt, in_=wl_d[k * 128:(k + 1) * 128, :])
                    wl_t.append(wlt)

            ones_t = cpool.tile([1, 128], dt_mm)
            nc.sync.dma_start(out=ones_t, in_=ones_d.rearrange("(a n) -> a n", a=1))
            bias_t = cpool.tile([1, O_SHARD], dt_mm)
            nc.sync.dma_start(out=bias_t, in_=b_d.rearrange("(a n) -> a n", a=1))

            # Carry state per m-tile: d = m_prev * (1 - s_prev) * alpha
            d_t = []
            for mi in range(MT):
                dt_ = cpool.tile([128, O_SHARD], f32, name=f"d{mi}")
                nc.vector.memset(dt_, 0.0)
                d_t.append(dt_)

            for t in range(T):
                ps = [psum_pool.tile([128, O_SHARD], f32, tag="ps",
                                     name=f"ps{t}_{mi}")
                      for mi in range(MT)]
                for k in range(KT):
                    xh = xpool.tile([128, B], dt_mm, tag="xh")
                    nc.sync.dma_start(out=xh, in_=xh_d[t, k * 128:(k + 1) * 128, :])
                    if split:
                        xl = xpool.tile([128, B], dt_mm, tag="xl")
                        nc.sync.dma_start(out=xl, in_=xl_d[t, k * 128:(k + 1) * 128, :])
                    for mi in range(MT):
                        sl = slice(mi * 128, (mi + 1) * 128)
                        nc.tensor.matmul(ps[mi], xh[:, sl], wh_t[k],
                                         start=(k == 0), stop=False)
                        if split:
                            nc.tensor.matmul(ps[mi], xh[:, sl], wl_t[k],
                                             start=False, stop=False)
                            nc.tensor.matmul(ps[mi], xl[:, sl], wh_t[k],
                                             start=False, stop=False)
                for mi in range(MT):
                    # bias add via K=1 matmul: ps += ones.T @ bias
                    nc.tensor.matmul(ps[mi], ones_t, bias_t,
                                     start=False, stop=True)
                for mi in range(MT):
                    m_sb = mpool.tile([128, O_SHARD], f32, tag="m")
                    nc.vector.tensor_add(out=m_sb, in0=ps[mi], in1=d_t[mi])
                    s_sb = spool.tile([128, O_SHARD], f32, tag="s")
                    nc.vector.tensor_scalar(out=s_sb, in0=m_sb, scalar1=M_TH,
                                            scalar2=None,
                                            op0=mybir.AluOpType.is_ge)
                    # u = (m < th) * alpha ; d = m * u
                    u_sb = upool.tile([128, O_SHARD], f32, tag="u")
                    nc.vector.tensor_scalar(out=u_sb, in0=m_sb, scalar1=M_TH,
                                            scalar2=ALPHA,
                                            op0=mybir.AluOpType.is_lt,
                                            op1=mybir.AluOpType.mult)
                    nc.vector.tensor_mul(out=d_t[mi], in0=m_sb, in1=u_sb)
                    sl = slice(mi * 128, (mi + 1) * 128)
                    nc.sync.dma_start(out=m_d[t, sl, :], in_=m_sb)
                    nc.sync.dma_start(out=s_d[t, sl, :], in_=s_sb)

    nc.compile()
    return nc


def _get_nc(mode: str):
    if mode not in _cache:
        _cache[mode] = _build_kernel(mode)
    return _cache[mode]


def _prepare_in_maps(x: np.ndarray, W: np.ndarray, b: np.ndarray, mode: str):
    xT = np.ascontiguousarray(x.transpose(0, 2, 1))  # [T, D_in, B]
    in_maps = []
    if mode == "f32r3":
        xh = _round_to_bits(xT, 11)
        xl = xT - xh
        Wh = _round_to_bits(W, 11)
        Wl = W - Wh
        for c in range(N_CORES):
            sl = slice(c * O_SHARD, (c + 1) * O_SHARD)
            in_maps.append({
                "xh": xh, "xl": xl,
                "wh": np.ascontiguousarray(Wh[sl, :].T),
                "wl": np.ascontiguousarray(Wl[sl, :].T),
                "bias": np.ascontiguousarray(b[sl]),
                "ones": np.ones(128, dtype=np.float32),
            })
    else:
        for c in range(N_CORES):
            sl = slice(c * O_SHARD, (c + 1) * O_SHARD)
            in_maps.append({
                "xh": xT,
                "wh": np.ascontiguousarray(W[sl, :].T),
                "bias": np.ascontiguousarray(b[sl]),
                "ones": np.ones(128, dtype=np.float32),
            })
    return in_maps


def kernel(x: np.ndarray, W: np.ndarray, b: np.ndarray):
    x = np.asarray(x, dtype=np.float32)
    W = np.asarray(W, dtype=np.float32)
    b = np.asarray(b, dtype=np.float32)
    nc = _get_nc(MODE)
    in_maps = _prepare_in_maps(x, W, b, MODE)
    res = run_bass_kernel_spmd(nc, in_maps, core_ids=list(range(N_CORES)))
    m = np.empty((T, B, D), dtype=np.float32)
    s = np.empty((T, B, D), dtype=np.float32)
    for c in range(N_CORES):
        sl = slice(c * O_SHARD, (c + 1) * O_SHARD)
        m[:, :, sl] = res.results[c]["m_out"]
        s[:, :, sl] = res.results[c]["s_out"]
    return (m, s)
